# revision 1
# baseline (speedup 1.0000x reference)
"""Trainium2 Bass kernel for a ResNet BasicBlock (dense CNN, sync-BN).

Reference computation (training-mode BN, batch stats over (N,H,W)):
    h = conv3x3(x, W1) * mask1            # structured channel pruning
    h = relu(bn(h, gamma1, beta1))
    h = conv3x3(h, W2) * mask2
    h = bn(h, gamma2, beta2)
    out = relu(h + x)                      # identity shortcut

Shapes: x [32, 256, 56, 56] f32, W [256, 256, 3, 3] f32.

Strategy: data-parallel over batch N across 8 NeuronCores (4 images per
core), weights replicated.  BN batch statistics are synchronized with a
tiny (2 KB) AllReduce of per-channel (sum, sum-of-squares) pairs.

Per-core layout:
  - Channels are split into two 128-partition halves (C=256 = 2*128).
  - Conv inputs live in SBUF as zero-padded 58x58 bf16 planes (row
    stride 58), so each of the 9 taps of the 3x3 conv is a plain offset
    shift: one matmul per (tap, ci-half) accumulating into PSUM.
  - Each image's 56 output rows are produced in 7 chunks of 8 rows
    (464 = 8*58 contiguous padded positions) so a chunk fits a PSUM
    bank; the interior (56 of 58 columns) is copied out with a strided
    AP, discarding the junk computed at the pad columns.
  - Masks are folded into the weights on the host (zero rows), so
    masked output channels are exactly zero everywhere.
"""

import numpy as np
import ml_dtypes

# ---- problem constants (hardcoded; kernel.py must be self-contained) ----
N_TOT, C, H, W = 32, 256, 56, 56
N_CORES = 8
NL = N_TOT // N_CORES          # images per core
PW = H + 2                     # padded row stride (58)
PLANE = PW * PW + 4            # padded plane floats + 4 spare for tap overreads
STRIP0 = PW + 1                # first interior output position (59)
CHUNK = 8 * PW                 # 464: 8 output rows per chunk
NCHUNK = 7                     # 7 chunks * 8 rows = 56 rows
HW = H * W                     # 3136
HALF_ROWS = 28                 # row granularity for x/out streaming DMAs
HALF_ELEMS = HALF_ROWS * W     # 1568
COUNT = N_TOT * HW             # sync-BN element count per channel
EPS = 1e-5

_BF16 = ml_dtypes.bfloat16

_cache = {}


def _pack_weights(W1, W2, mask1, mask2):
    """-> [128, 72*128] bf16: [i, (conv,ky,kx,ci,co), o] with masks folded."""
    Wm = np.stack([W1 * mask1[:, None, None, None],
                   W2 * mask2[:, None, None, None]]).astype(np.float32)
    # [conv, O, I, 3, 3] -> [conv, co, o, ci, i, ky, kx]
    Wr = Wm.reshape(2, 2, 128, 2, 128, 3, 3)
    # -> [conv, ky, kx, ci, co, i, o]
    A = Wr.transpose(0, 5, 6, 3, 1, 4, 2)
    # -> [i, t, o] -> [128, 72*128]
    B = np.ascontiguousarray(A.transpose(5, 0, 1, 2, 3, 4, 6)).reshape(128, 72 * 128)
    return B.astype(_BF16)


def _t_index(conv, ky, kx, ci, co):
    return co + 2 * (ci + 2 * (kx + 3 * (ky + 3 * conv)))


def _pack_aff(gamma1, beta1, gamma2, beta2):
    cols = [gamma1, beta1, gamma2, beta2]
    out = np.empty((128, 8), np.float32)
    for k, v in enumerate(cols):
        v = np.asarray(v, np.float32).reshape(2, 128)
        out[:, 2 * k] = v[0]
        out[:, 2 * k + 1] = v[1]
    return out


def _build():
    import concourse.bass as bass_mod
    import concourse.bacc as bacc
    import concourse.mybir as mybir
    import concourse.tile as tile

    f32 = mybir.dt.float32
    bf16 = mybir.dt.bfloat16
    AX = mybir.AxisListType
    ALU = mybir.AluOpType
    AF = mybir.ActivationFunctionType

    nc = bacc.Bacc("TRN2", target_bir_lowering=False, debug=False,
                   num_devices=N_CORES)

    x_d = nc.dram_tensor("x", [NL, C, H, W], f32, kind="ExternalInput")
    wt_d = nc.dram_tensor("wt", [128, 72 * 128], bf16, kind="ExternalInput")
    aff_d = nc.dram_tensor("aff", [128, 8], f32, kind="ExternalInput")
    out_d = nc.dram_tensor("out", [NL, C, H, W], f32, kind="ExternalOutput")

    groups = [list(range(N_CORES))]

    def interior(tile_ap, base, nrows):
        """[128, nrows, 56] strided view (row stride PW) starting at `base`."""
        v = tile_ap[:, base:base + nrows * PW].rearrange(
            "p (r c) -> p r c", c=PW)
        return v[:, :, 0:W]

    with tile.TileContext(nc) as tc:
        import contextlib
        with contextlib.ExitStack() as ctx:
            const = ctx.enter_context(tc.tile_pool(name="const", bufs=1))
            psum = ctx.enter_context(tc.tile_pool(name="psum", bufs=6, space="PSUM"))
            xst = ctx.enter_context(tc.tile_pool(name="xst", bufs=2))
            otp = ctx.enter_context(tc.tile_pool(name="otp", bufs=2))
            sqp = ctx.enter_context(tc.tile_pool(name="sqp", bufs=2))

            wt_sb = const.tile([128, 72 * 128], bf16, tag="wt", name="wt")
            nc.sync.dma_start(wt_sb[:], wt_d[:])
            aff_sb = const.tile([128, 8], f32, tag="aff", name="aff")
            nc.sync.dma_start(aff_sb[:], aff_d[:])

            # ---- cross-core stats exchange plumbing (SBUF remote DMA) ----
            # Each of the 4 BN-stat exchanges broadcasts this core's [128,2]
            # (sum, sumsq) to all 7 peers with XOR-relative dests; slot d of
            # the receive tile gets the copy from core (me ^ d).  Hardware
            # remote sems count arrivals (2 per transfer -> wait >= 14).
            rsem = [nc.alloc_semaphore(f"rst{i}") for i in range(4)]
            lsem = nc.alloc_semaphore("lst")
            _gp_prev = [None]
            # waits that the single-core scheduling simulator cannot satisfy
            # (remote increments); attached to the instructions after the
            # TileContext exits, before nc.compile()
            deferred_waits = []

            def gp_order(bi):
                if _gp_prev[0] is not None:
                    bass_mod._add_dep_helper(bi.ins, _gp_prev[0].ins,
                                             sync=False,
                                             reason="stats-exchange order")
                _gp_prev[0] = bi
                return bi

            # register the entry barrier (prelude AllGather increments
            # _bir_kernel_barrier_sem once every core has entered the NEFF)
            nc._bir_kernel_barrier_sem_replica_groups.extend(
                set(g) for g in groups)

            def defer_wait(bi, sem, val):
                # reserve the wait slot with an always-satisfied threshold so
                # the scheduling simulator passes; patched to `val` later
                bi._wait_ge(sem, 0)
                deferred_waits.append((bi, sem, val))
                return bi

            # sems persist across NEFF executions: clear them as soon as all
            # cores have entered (peers send >100us later, after conv1)
            for i, s in enumerate(rsem + [lsem]):
                cl = gp_order(nc.gpsimd.sem_clear(s))
                if i == 0:
                    defer_wait(cl, nc._bir_kernel_barrier_sem,
                               nc.bir_kernel_barrier_sem_inc)

            # persistent per-image planes
            x_pad = [[const.tile([128, PLANE], bf16, tag=f"xp{j}_{n}", name=f"xp{j}_{n}")
                      for n in range(NL)] for j in range(2)]
            h1_pad = [[const.tile([128, PLANE], bf16, tag=f"h1{j}_{n}", name=f"h1{j}_{n}")
                       for n in range(NL)] for j in range(2)]
            h2 = [[const.tile([128, HW], bf16, tag=f"h2{j}_{n}", name=f"h2{j}_{n}")
                   for n in range(NL)] for j in range(2)]

            # zero the non-interior positions of every padded plane:
            #  (a) [0, 59)  (b) pad-column pairs  (c) [3307, PLANE)
            for planes in (x_pad, h1_pad):
                for j in range(2):
                    for n in range(NL):
                        t = planes[j][n]
                        nc.vector.memset(t[:, 0:STRIP0], 0.0)
                        pairs = t[:, 2 * PW - 1:2 * PW - 1 + 56 * PW].rearrange(
                            "p (r c) -> p r c", c=PW)[:, :, 0:2]
                        nc.vector.memset(pairs, 0.0)
                        nc.vector.memset(t[:, STRIP0 + 56 * PW:PLANE], 0.0)

            # stats accumulators: one column per (image, chunk)
            acc = {(b, s, j): const.tile([128, NL * NCHUNK], f32,
                                         tag=f"acc{b}{s}{j}", name=f"acc{b}{s}{j}")
                   for b in (1, 2) for s in ("s", "q") for j in range(2)}

            # ---- head: stream x in, cast-scatter into padded bf16 planes ----
            for n in range(NL):
                for j in range(2):
                    for rh in range(2):
                        r0 = rh * HALF_ROWS
                        xs = xst.tile([128, HALF_ELEMS], f32, tag="xs", name="xs")
                        nc.sync.dma_start(
                            xs[:], x_d[n, j * 128:(j + 1) * 128, r0:r0 + HALF_ROWS, :])
                        dst = interior(x_pad[j][n], (r0 + 1) * PW + 1, HALF_ROWS)
                        src = xs[:, :].rearrange("p (r c) -> p r c", c=W)
                        nc.vector.tensor_copy(dst, src)

            # ---- conv + stats emission helper (one output half) ----
            def conv(conv_i, src_planes, dst_j_n_base, bn_i, j):
                """src_planes[ci][n]: padded bf16 planes; writes dst via
                dst_j_n_base(j, n, k) -> (tile, interior_view) and stats."""
                for n in range(NL):
                    if True:
                        for k in range(NCHUNK):
                            pt = psum.tile([128, 8 * W], f32, tag="ps", name="ps")
                            idx = 0
                            for ci in range(2):
                                for ky in range(3):
                                    for kx in range(3):
                                        t = _t_index(conv_i, ky, kx, ci, j)
                                        dq = (ky - 1) * PW + (kx - 1)
                                        off = STRIP0 + CHUNK * k + dq
                                        rhs = src_planes[ci][n][
                                            :, off:off + CHUNK].rearrange(
                                            "p (r c) -> p r c", c=PW)[:, :, 0:W]
                                        nc.tensor.matmul(
                                            pt[:],
                                            wt_sb[:, t * 128:(t + 1) * 128],
                                            rhs,
                                            start=(idx == 0), stop=(idx == 17))
                                        idx += 1
                            src_int = pt[:, 0:8 * W].rearrange(
                                "p (r c) -> p r c", c=W)
                            dst_int = dst_j_n_base(j, n, k)
                            col = n * NCHUNK + k
                            # copy + sum on ACT; square + sumsq also on ACT so
                            # the DVE stream stays free for AR-gated work
                            nc.scalar.activation(
                                dst_int, src_int, AF.Copy,
                                accum_out=acc[(bn_i, "s", j)][:, col:col + 1])
                            sq = sqp.tile([128, 8 * W], f32, tag="sq", name="sq")
                            last = nc.scalar.activation(
                                sq[:, :].rearrange("p (r c) -> p r c", c=W),
                                dst_int, AF.Square,
                                accum_out=acc[(bn_i, "q", j)][:, col:col + 1])
                return last

            # ---- BN stats: local reduce + send to all peers (one half).
            # Emitted right after the conv that produced the stats so the
            # transfer latency overlaps the other half's conv compute. ----
            def bn_stats_send(bn_i, j, ex):
                sfx = f"{bn_i}{j}"
                packed = const.tile([128, 2], f32, tag=f"pk{sfx}", name=f"pk{sfx}")
                nc.vector.tensor_reduce(
                    packed[:, 0:1], acc[(bn_i, "s", j)][:], axis=AX.X, op=ALU.add)
                nc.vector.tensor_reduce(
                    packed[:, 1:2], acc[(bn_i, "q", j)][:], axis=AX.X, op=ALU.add)
                rv = const.tile([128, 16], f32, tag=f"rv{ex}", name=f"rv{ex}")
                nc.vector.tensor_copy(rv[:, 0:2], packed[:])   # own slot (d=0)
                for d in range(1, 8):
                    rd = [None] * 8
                    rd[d] = (0, d)
                    gp_order(nc.gpsimd.remote_dma_broadcast(
                        rv[:, 2 * d:2 * d + 2], packed[:],
                        remote_sem=rsem[ex], local_sem=lsem, rdests=rd))
                gp_order(nc.gpsimd.trigger_dma(count=None))
                return rv

            # ---- wait for all 8 contributions, sum slots -> global stats.
            # MUST be emitted after every conv-phase op of DVE's stream: any
            # long-blocked op sitting ahead of conv work in an engine's
            # in-order stream stalls cross-engine waits that point past it.
            def bn_stats_recv(bn_i, j, ex, rv, after):
                sfx = f"{bn_i}{j}"
                gl = const.tile([128, 2], f32, tag=f"gl{sfx}", name=f"gl{sfx}")
                red = nc.vector.tensor_reduce(
                    gl[:], rv[:, 0:16].rearrange("p (s c) -> p c s", c=2),
                    axis=AX.X, op=ALU.add)
                defer_wait(red, rsem[ex], 14)
                # pin the whole arrival-gated chain behind the conv phase so
                # the scheduler cannot interleave it into engine streams that
                # conv-era waits threshold against
                bass_mod._add_dep_helper(red.ins, after.ins, sync=True,
                                         reason="recv after conv phase")
                return gl

            # ---- BN affine from global stats — DVE-only (no ACT ops, so
            # waiting on the collective never blocks ACT's copy stream).
            # rsqrt via the fast-inverse-sqrt bit trick + 2 Newton steps. ----
            def bn_affine_finish(bn_i, j, gl, g_col, b_col):
                sfx = f"{bn_i}{j}"
                mean = const.tile([128, 1], f32, tag=f"mean{sfx}", name=f"mean{sfx}")
                nc.vector.tensor_scalar_mul(mean[:], gl[:, 0:1], 1.0 / COUNT)
                var = const.tile([128, 1], f32, tag=f"var{sfx}", name=f"var{sfx}")
                nc.vector.tensor_tensor(var[:], mean[:], mean[:], ALU.mult)
                nc.vector.scalar_tensor_tensor(
                    var[:], gl[:, 1:2], 1.0 / COUNT, var[:],
                    ALU.mult, ALU.subtract)
                nc.vector.tensor_scalar_add(var[:], var[:], EPS)
                y = const.tile([128, 1], f32, tag=f"y{sfx}", name=f"y{sfx}")
                vh = const.tile([128, 1], f32, tag=f"vh{sfx}", name=f"vh{sfx}")
                tmp = const.tile([128, 1], f32, tag=f"tm{sfx}", name=f"tm{sfx}")
                iv = var[:].bitcast(mybir.dt.int32)
                yi = y[:].bitcast(mybir.dt.int32)
                nc.vector.tensor_scalar(yi, iv, 1, None, ALU.arith_shift_right)
                nc.vector.tensor_scalar(yi, yi, -1, None, ALU.bitwise_xor)
                nc.vector.tensor_scalar(yi, yi, 0x5f3759df + 1, None, ALU.add)
                nc.vector.tensor_scalar_mul(vh[:], var[:], 0.5)
                for _ in range(2):
                    nc.vector.tensor_tensor(tmp[:], y[:], y[:], ALU.mult)
                    nc.vector.tensor_tensor(tmp[:], tmp[:], vh[:], ALU.mult)
                    nc.vector.tensor_scalar(tmp[:], tmp[:], -1.0, 1.5,
                                            ALU.mult, ALU.add)
                    nc.vector.tensor_tensor(y[:], y[:], tmp[:], ALU.mult)
                sc = const.tile([128, 1], f32, tag=f"sc{sfx}", name=f"sc{sfx}")
                nc.vector.tensor_tensor(sc[:], aff_sb[:, g_col + j:g_col + j + 1],
                                        y[:], ALU.mult)
                bi = const.tile([128, 1], f32, tag=f"bi{sfx}", name=f"bi{sfx}")
                nc.vector.tensor_tensor(bi[:], mean[:], sc[:], ALU.mult)
                nc.vector.tensor_tensor(bi[:], aff_sb[:, b_col + j:b_col + j + 1],
                                        bi[:], ALU.subtract)
                return sc, bi

            def h1_dst(j, n, k):
                return interior(h1_pad[j][n], (1 + 8 * k) * PW + 1, 8)

            def h2_dst(j, n, k):
                return h2[j][n][:, 8 * k * W:(8 * k + 8) * W].rearrange(
                    "p (r c) -> p r c", c=W)

            def tail(j, s2, b2, use_act):
                # out = relu(s2*h2 + b2 + x); x from resident bf16 planes.
                # use_act=False → all-DVE (so waiting on AR2 never blocks
                # ACT's copy stream while the other half is still convolving).
                # use_act=True (last phase, ACT idle) → relu on ACT, pipelined.
                for m, (n, rh) in enumerate((n, rh) for n in range(NL)
                                            for rh in range(2)):
                    r0 = rh * HALF_ROWS
                    xv = interior(x_pad[j][n], (r0 + 1) * PW + 1, HALF_ROWS)
                    h2v = h2[j][n][:, r0 * W:r0 * W + HALF_ELEMS].rearrange(
                        "p (r c) -> p r c", c=W)
                    pool = otp if m % 2 == 0 else xst
                    ot = pool.tile([128, HALF_ELEMS], f32,
                                   tag="ot" if m % 2 == 0 else "xs", name="ot")
                    otv = ot[:, :].rearrange("p (r c) -> p r c", c=W)
                    nc.vector.scalar_tensor_tensor(
                        otv, h2v, s2[:], xv, ALU.mult, ALU.add)
                    if use_act:
                        nc.scalar.activation(ot[:], ot[:], AF.Relu,
                                             bias=b2[:], scale=1.0)
                    else:
                        nc.vector.tensor_scalar(ot[:], ot[:], b2[:], 0.0,
                                                ALU.add, ALU.max)
                    nc.sync.dma_start(
                        out_d[n, j * 128:(j + 1) * 128, r0:r0 + HALF_ROWS, :],
                        ot[:])

            # ---- phase schedule: j-outer; stats are SENT right after the
            # conv half that produced them (transfer overlaps the other
            # half's conv), but RECEIVED only after all conv emission so no
            # arrival-gated op sits ahead of conv work in an engine stream.
            conv(0, x_pad, h1_dst, 1, 0)           # conv1 half 0
            rv1_0 = bn_stats_send(1, 0, 0)
            c1_last = conv(0, x_pad, h1_dst, 1, 1)  # conv1 half 1
            rv1_1 = bn_stats_send(1, 1, 1)
            gl1_0 = bn_stats_recv(1, 0, 0, rv1_0, c1_last)
            s1_0, b1_0 = bn_affine_finish(1, 0, gl1_0, 0, 2)
            gl1_1 = bn_stats_recv(1, 1, 1, rv1_1, c1_last)  # peer-skew wait
            s1_1, b1_1 = bn_affine_finish(1, 1, gl1_1, 0, 2)
            # BN1 apply + relu in place (ACT) — n-major so conv2 image 0
            # unblocks first
            for n in range(NL):
                for j, (s1, b1) in ((0, (s1_0, b1_0)), (1, (s1_1, b1_1))):
                    v = interior(h1_pad[j][n], STRIP0, H)
                    nc.scalar.activation(v, v, AF.Relu,
                                         bias=b1[:], scale=s1[:])
            conv(1, h1_pad, h2_dst, 2, 0)          # conv2 half 0
            rv2_0 = bn_stats_send(2, 0, 2)
            c2_last = conv(1, h1_pad, h2_dst, 2, 1)  # conv2 half 1
            rv2_1 = bn_stats_send(2, 1, 3)
            gl2_0 = bn_stats_recv(2, 0, 2, rv2_0, c2_last)
            s2_0, b2_0 = bn_affine_finish(2, 0, gl2_0, 4, 6)
            tail(0, s2_0, b2_0, use_act=True)      # overlaps half-1 peer skew
            gl2_1 = bn_stats_recv(2, 1, 3, rv2_1, c2_last)
            s2_1, b2_1 = bn_affine_finish(2, 1, gl2_1, 4, 6)
            tail(1, s2_1, b2_1, use_act=True)

    # patch the reserved wait slots to their real thresholds now that
    # scheduling is done (the single-core scheduling simulator cannot
    # satisfy remote increments)
    for bi, sem, val in deferred_waits:
        patched = False
        for w in bi.ins.sync_info.on_wait:
            if w.id == sem.num and w.wait_value == 0:
                w.wait_value = val
                patched = True
                break
        assert patched, f"deferred wait not found on {bi.ins.name}"

    nc.compile()
    return nc


def kernel(x, W1, W2, gamma1, beta1, gamma2, beta2, mask1, mask2,
           _trace=False, _trace_kwargs=None):
    from concourse.bass_utils import run_bass_kernel_spmd

    if "nc" not in _cache:
        _cache["nc"] = _build()
    nc = _cache["nc"]

    wt = _pack_weights(np.asarray(W1, np.float32), np.asarray(W2, np.float32),
                       np.asarray(mask1, np.float32), np.asarray(mask2, np.float32))
    aff = _pack_aff(gamma1, beta1, gamma2, beta2)
    x = np.ascontiguousarray(np.asarray(x, np.float32))

    in_maps = [{"x": x[i * NL:(i + 1) * NL], "wt": wt, "aff": aff}
               for i in range(N_CORES)]
    kw = {}
    if _trace:
        kw = dict(trace=True, **(_trace_kwargs or {}))
    res = run_bass_kernel_spmd(nc, in_maps, core_ids=list(range(N_CORES)), **kw)
    out = np.concatenate([res.results[i]["out"] for i in range(N_CORES)], axis=0)
    _cache["last_results"] = res
    return out

